# revision 1
# baseline (speedup 1.0000x reference)
"""DenseMaskPredictor Trainium2 kernel.

out[n] = paste(sigmoid(mask_output[n, cls[n]]), bbox[n]) onto a 768x768 canvas,
zero outside the box (bilinear, zero-padded sampling).

Math: the bilinear paste is separable:
    out_n[y, x] = sum_ij Wy[y,i] * probs_n[i,j] * Wx[x,j]
with W*[s, k] = relu(1 - a*|s - c_k|), c_k = s0' + (k+0.5)*(s1-s0)/28,
a = 28/(s1-s0). This reproduces the reference's zero-padded bilinear exactly,
including boundary semantics (weights vanish outside the box; index clipping
never matters because clipped indices carry zero weight).

Device plan (per core, 16 instances as 4 groups of 4; instance b-of-group lives
at partition block 32*b of every tile):
  - per-instance scalars (s0', (s1-s0)/28, -a per axis; validity folded into
    s0' as a -1e9 penalty) replicated over each 32-partition block by one tiny
    matmul against a block-diagonal 0/1 matrix.
  - WyT/WxT [28(+4 pad), 768] weight tiles: ScalarE Abs(iota - c) with a
    per-partition bias, then two dual-op tensor_scalar ops on VectorE.
    Pad rows k>=28 get c ~ 1e9 so their weight is exactly 0.
  - class-mask gather via one indirect DMA (row index 80n + clip(cls)),
    per-instance rearrange to [28, 28], sigmoid on ScalarE.
  - V[j, y] = sum_i probs[i,j] WyT[i,y]: K=28 matmuls at row/col tile
    position (32b, 32b) -- 4 instances run concurrently on the PE array.
  - out[y, x] = sum_j V[j, ytile] WxT[j, x]: 12 matmuls/instance (N=512+256)
    in float32r (single-pass fp32; plain fp32 is two HW passes), pairs of
    instances interleaved across PE row groups.
  - PSUM evacuated by alternating ScalarE/VectorE [128, 768] copies into
    [128, 4*768] staging tiles; one 768KB HWDGE DMA per instance-pair per
    y-tile to DRAM (~330 GB/s sustained; the 37.75MB/core output write is
    the roofline).

Data-parallel over N=128 instances across 8 cores (16 each). No collectives.
Measured: ~139us HW exec, rel err ~2.4e-4 vs the fp32 reference (float32r
multiply noise; set MM_FP32R=False for ~4e-6 at ~1.4x the time).
"""

import os
import sys

import numpy as np

for _p in ("/opt/trn_rl_repo",):
    if _p not in sys.path and os.path.isdir(_p):
        sys.path.insert(0, _p)

N_FULL = 128
N_CORES = 8
N_LOC = N_FULL // N_CORES  # 16 instances per core
C = 80
M = 28
H = W = 768
NUM_VALID = 80
GROUPS = N_LOC // 4  # groups of 4 instances
MM_FP32R = True  # single-pass fp32 matmuls (2x fewer PE passes than fp32)

F32 = None  # set on import of mybir inside _build


def _emit(tc, nc, masks, cls, bbox, out):
    import concourse.bass as bass
    from concourse import mybir

    f32 = mybir.dt.float32
    i32 = mybir.dt.int32
    AF = mybir.ActivationFunctionType
    OP = mybir.AluOpType
    ctx = tc._emit_ctx  # ExitStack supplied by caller

    # float32r: single-pass fp32 matmul (vs the default 2-pass hi/lo split).
    # Tiles feeding those matmuls must be *produced* as float32r (the BIR
    # verifier requires rounded producers). Restricted to the out-stage
    # matmuls: fp32r is ISA-invalid for nonzero column tile positions (the V
    # matmuls), and the g-row broadcast needs full fp32 anyway.
    f32r = mybir.dt.float32r if MM_FP32R else f32
    mmr = lambda ap: ap

    const = ctx.enter_context(tc.tile_pool(name="const", bufs=1))
    small = ctx.enter_context(tc.tile_pool(name="small", bufs=1))
    gpool = ctx.enter_context(tc.tile_pool(name="gpool", bufs=2))
    wpool = ctx.enter_context(tc.tile_pool(name="wpool", bufs=2))
    vpool = ctx.enter_context(tc.tile_pool(name="vpool", bufs=2))
    ppool = ctx.enter_context(tc.tile_pool(name="ppool", bufs=2))
    stage = ctx.enter_context(tc.tile_pool(name="stage", bufs=6))
    # g_bc and v_ps share one 2-bank slot (sequential within a group); the
    # remaining 6 banks hold three [128, 768] out tiles (2 banks each), each
    # filled by two matmuls and evacuated by one merged copy.
    ps_m = ctx.enter_context(tc.tile_pool(name="ps_m", bufs=1, space="PSUM"))
    ps_o = ctx.enter_context(tc.tile_pool(name="ps_o", bufs=3, space="PSUM"))

    # ---------------- constants ----------------
    iota_i = const.tile([128, W], i32)
    nc.gpsimd.iota(iota_i[:, :], pattern=[[1, W]], channel_multiplier=0)
    iota_f = const.tile([128, W], f32)
    nc.vector.tensor_copy(iota_f[:, :], iota_i[:, :])

    p_col = const.tile([128, 1], i32)
    nc.gpsimd.iota(p_col[:, :], pattern=[[0, 1]], channel_multiplier=1)
    k_i = const.tile([128, 1], i32)
    nc.vector.tensor_scalar(k_i[:, :], p_col[:, :], 31, None, op0=OP.bitwise_and)
    k_f = const.tile([128, 1], f32)
    nc.vector.tensor_copy(k_f[:, :], k_i[:, :])
    # kcol = k + 0.5 for k < 28, else huge (pad rows produce zero weight)
    k_hi = const.tile([128, 1], f32)
    nc.vector.tensor_scalar(k_hi[:, :], k_f[:, :], 27.5, 0.0, op0=OP.subtract, op1=OP.max)
    nc.vector.tensor_scalar(k_hi[:, :], k_hi[:, :], 4.0e8, None, op0=OP.mult)
    kcol = const.tile([128, 1], f32)
    nc.vector.tensor_scalar(kcol[:, :], k_f[:, :], 0.5, None, op0=OP.add)
    nc.vector.tensor_add(kcol[:, :], kcol[:, :], k_hi[:, :])
    negk = const.tile([128, 1], f32)
    nc.vector.tensor_scalar(negk[:, :], kcol[:, :], -1.0, None, op0=OP.mult)

    # block-diagonal broadcast matrix: blkT[p, q] = 1 iff q//32 == p%32.
    # Rows used as lhsT are 32g..32g+3 (p%32 = b in 0..3), mapping instance b
    # of the group onto output partitions 32b..32b+31.
    q5_i = const.tile([128, 128], i32)
    nc.vector.tensor_scalar(q5_i[:, :], iota_i[:, :128], 5, None, op0=OP.arith_shift_right)
    q5_f = const.tile([128, 128], f32)
    nc.vector.tensor_copy(q5_f[:, :], q5_i[:, :])
    # blkT = 1 iff q5 == k (both small ints): 1 - min((q5-k)^2, 1)
    bdiff = const.tile([128, 128], f32)
    nc.vector.tensor_scalar(bdiff[:, :], q5_f[:, :], k_f[:, :], None, op0=OP.subtract)
    bsq = const.tile([128, 128], f32)
    nc.vector.tensor_mul(bsq[:, :], bdiff[:, :], bdiff[:, :])
    blkT = const.tile([128, 128], f32)
    nc.vector.tensor_scalar(blkT[:, :], bsq[:, :], 1.0, -1.0, op0=OP.min, op1=OP.mult)
    nc.vector.tensor_scalar(blkT[:, :], blkT[:, :], 1.0, None, op0=OP.add)

    # ---------------- input prep ----------------
    cls16 = small.tile([N_LOC, 1], i32)
    nc.sync.dma_start(cls16[:, :], cls[:, :])
    iota80 = small.tile([N_LOC, 1], i32)
    nc.gpsimd.iota(iota80[:, :], pattern=[[0, 1]], channel_multiplier=C)
    cls_cl = small.tile([N_LOC, 1], i32)
    nc.vector.tensor_scalar(cls_cl[:, :], cls16[:, :], 0, C - 1, op0=OP.max, op1=OP.min)
    off16 = small.tile([N_LOC, 1], i32)
    nc.vector.tensor_add(off16[:, :], cls_cl[:, :], iota80[:, :])

    sel_all = small.tile([N_LOC, M * M], f32)
    nc.gpsimd.indirect_dma_start(
        out=sel_all[:, :],
        out_offset=None,
        in_=masks.rearrange("n c h w -> (n c) (h w)"),
        in_offset=bass.IndirectOffsetOnAxis(ap=off16[:, :], axis=0),
    )

    bbox_sp = small.tile([128, 4], f32)
    nc.vector.memset(bbox_sp[:, 0:2], 0.0)
    nc.vector.memset(bbox_sp[:, 2:4], 4.0)
    cls_sp = small.tile([128, 1], i32)
    nc.vector.memset(cls_sp[:, :], 0)
    for g in range(GROUPS):
        nc.sync.dma_start(bbox_sp[32 * g : 32 * g + 4, :], bbox[4 * g : 4 * g + 4, :])
        nc.sync.dma_start(cls_sp[32 * g : 32 * g + 4, :], cls[4 * g : 4 * g + 4, :])

    # validity penalty: 0 if 0 <= cls < NUM_VALID else <= -1e9 (pushes g out of range)
    clsf = small.tile([128, 1], f32)
    nc.vector.tensor_copy(clsf[:, :], cls_sp[:, :])
    u_lo = small.tile([128, 1], f32)
    nc.vector.tensor_scalar(u_lo[:, :], clsf[:, :], -1.0, 0.0, op0=OP.mult, op1=OP.max)
    u_hi = small.tile([128, 1], f32)
    nc.vector.tensor_scalar(
        u_hi[:, :], clsf[:, :], float(NUM_VALID - 1), 0.0, op0=OP.subtract, op1=OP.max
    )
    pen = small.tile([128, 1], f32)
    nc.vector.tensor_add(pen[:, :], u_lo[:, :], u_hi[:, :])
    nc.vector.tensor_scalar(pen[:, :], pen[:, :], -1.0e9, None, op0=OP.mult)

    # Per-axis, per-instance scalars packed into vals_sp columns (at spread
    # rows 32g+b): for axis q in (x=0, y=1): col 3q+0 = s0' (origin, incl. the
    # validity penalty), 3q+1 = ra = (s1-s0)/28, 3q+2 = -a = -28/(s1-s0).
    # Weight rows are then w[p, s] = relu(1 - a*|s - (s0' + kcol[p]*ra)|),
    # built per group after a tiny matmul replicates the scalars over each
    # instance's 32-partition block.
    vals_sp = small.tile([128, 6], f32)
    for q, (c0, c1) in enumerate(((0, 2), (1, 3))):  # x: (x0, x1), y: (y0, y1)
        dx = small.tile([128, 1], f32, name=f"dx{c0}")
        nc.vector.tensor_sub(dx[:, :], bbox_sp[:, c1 : c1 + 1], bbox_sp[:, c0 : c0 + 1])
        nc.vector.tensor_scalar(
            vals_sp[:, 3 * q + 1 : 3 * q + 2], dx[:, :], 1.0 / float(M), None, op0=OP.mult
        )
        rx = small.tile([128, 1], f32, name=f"rx{c0}")
        nc.vector.reciprocal(rx[:, :], dx[:, :])
        nc.vector.tensor_scalar(
            vals_sp[:, 3 * q + 2 : 3 * q + 3], rx[:, :], -float(M), None, op0=OP.mult
        )
        x0p = small.tile([128, 1], f32, name=f"x0p{c0}")
        nc.vector.tensor_scalar(x0p[:, :], bbox_sp[:, c0 : c0 + 1], -0.5, None, op0=OP.add)
        nc.vector.tensor_add(vals_sp[:, 3 * q : 3 * q + 1], x0p[:, :], pen[:, :])

    CH = ((0, 512), (512, 256))  # x-chunks (start, len), N<=512 per matmul

    # ---------------- per-group pipeline ----------------
    for g in range(GROUPS):
        # class-mask probabilities [i, j] at block 32b
        probs_pre = ppool.tile([128, M], f32, tag="probs_pre")
        nc.vector.memset(probs_pre[:, :], 0.0)
        for b in range(4):
            n = 4 * g + b
            nc.gpsimd.dma_start(
                probs_pre[32 * b : 32 * b + M, :],
                sel_all[n : n + 1, :].rearrange("p (i j) -> p i j", i=M),
            )
        probs = ppool.tile([128, M], f32, tag="probs")
        nc.scalar.activation(probs[:, :], probs_pre[:, :], AF.Sigmoid)

        # replicate the 6 per-instance scalars across each 32-partition block
        vals_ps = ps_m.tile([128, 8], f32, tag="ps_misc", name="vals_ps")
        nc.tensor.matmul(
            out=vals_ps[:, :6],
            lhsT=blkT[32 * g : 32 * g + 4, :],
            rhs=vals_sp[32 * g : 32 * g + 4, :],
            start=True,
            stop=True,
            tile_position=(32 * g, 0),
        )
        vals = ppool.tile([128, 6], f32, tag="vals")
        nc.scalar.copy(vals[:, :], vals_ps[:, :6])

        # interpolation weights: w[p, s] = relu(1 - a*|s - (s0' + kcol*ra)|)
        # (positive weights on both axes; pad rows k>=28 get huge c -> w=0)
        w_tiles = []
        for ax_idx, q in enumerate((1, 0)):  # y first, then x
            c_col = ppool.tile([128, 1], f32, tag=f"c_col{ax_idx}")
            nc.vector.tensor_scalar(
                c_col[:, :],
                kcol[:, :],
                vals[:, 3 * q + 1 : 3 * q + 2],
                vals[:, 3 * q : 3 * q + 1],
                op0=OP.mult,
                op1=OP.add,
            )
            negc = ppool.tile([128, 1], f32, tag=f"negc{ax_idx}")
            nc.vector.tensor_scalar(negc[:, :], c_col[:, :], -1.0, None, op0=OP.mult)
            d_t = gpool.tile([128, W], f32, tag=f"d_t{ax_idx}")
            nc.scalar.activation(d_t[:, :], iota_f[:, :], AF.Abs, bias=negc[:, :])
            w1_t = gpool.tile([128, W], f32, tag=f"w1_t{ax_idx}")
            nc.vector.tensor_scalar(
                w1_t[:, :],
                d_t[:, :],
                vals[:, 3 * q + 2 : 3 * q + 3],
                1.0,
                op0=OP.mult,
                op1=OP.add,
            )
            w_t = wpool.tile([128, W], f32r if ax_idx == 1 else f32, tag=f"w{ax_idx}")
            nc.vector.tensor_scalar(w_t[:, :], w1_t[:, :], 0.0, None, op0=OP.max)
            w_tiles.append(w_t)
        w_y, w_x = w_tiles

        # V[j, y] = sum_i probs[i, j] * WyT[i, y]
        v_ps = ps_m.tile([128, W], f32, tag="ps_misc", name="v_ps")
        for (c0, cn) in CH:
            for b in range(4):
                nc.tensor.matmul(
                    out=v_ps[32 * b : 32 * b + M, c0 : c0 + cn],
                    lhsT=mmr(probs[32 * b : 32 * b + M, :]),
                    rhs=mmr(w_y[32 * b : 32 * b + M, c0 : c0 + cn]),
                    start=True,
                    stop=True,
                    tile_position=(32 * b, 32 * b),
                )
        v_sb = vpool.tile([128, W], f32r, tag="v_sb")
        for b in range(4):
            nc.scalar.copy(v_sb[32 * b : 32 * b + M, :], v_ps[32 * b : 32 * b + M, :])

        # out[y, x] = sum_j V[j, y] * WxT[j, x]  (negations cancel)
        for t in range(6):
            st = stage.tile([128, 4 * W], f32, tag="st")
            for pair in range(2):
                o_tiles = []
                for j in range(2):
                    o_ps = ps_o.tile([128, W], f32, tag="o_ps", name=f"o_ps{j}")
                    o_tiles.append(o_ps)
                # interleave the two instances' matmuls (different PE row
                # groups run concurrently)
                for (c0, cn) in CH:
                    for j in range(2):
                        b = 2 * pair + j
                        nc.tensor.matmul(
                            out=o_tiles[j][:, c0 : c0 + cn],
                            lhsT=mmr(
                                v_sb[32 * b : 32 * b + M, t * 128 : (t + 1) * 128]
                            ),
                            rhs=mmr(w_x[32 * b : 32 * b + M, c0 : c0 + cn]),
                            start=True,
                            stop=True,
                            tile_position=(32 * b, 0),
                        )
                for j in range(2):
                    b = 2 * pair + j
                    dst = st[:, b * W : (b + 1) * W]
                    if (t + b) % 2 == 0:
                        nc.scalar.copy(dst, o_tiles[j][:, :])
                    else:
                        nc.vector.tensor_copy(dst, o_tiles[j][:, :])
                # half-staging DMA right after this pair's evacuations so the
                # DMA pipe never waits for the whole 4-instance tile
                nc.sync.dma_start(
                    out[
                        4 * g + 2 * pair : 4 * g + 2 * pair + 2,
                        t * 128 : (t + 1) * 128,
                        :,
                    ].rearrange("n y x -> y n x"),
                    st[:, 2 * pair * W : (2 * pair + 2) * W],
                )


def _build_program():
    import concourse.tile as tile
    from concourse import bacc, mybir
    from contextlib import ExitStack

    f32 = mybir.dt.float32
    i32 = mybir.dt.int32

    nc = bacc.Bacc("TRN2", target_bir_lowering=False, debug=False)
    masks = nc.dram_tensor("masks", [N_LOC, C, M, M], f32, kind="ExternalInput").ap()
    cls = nc.dram_tensor("cls", [N_LOC, 1], i32, kind="ExternalInput").ap()
    bbox = nc.dram_tensor("bbox", [N_LOC, 4], f32, kind="ExternalInput").ap()
    out = nc.dram_tensor("out", [N_LOC, H, W], f32, kind="ExternalOutput").ap()

    with tile.TileContext(nc) as tc:
        with ExitStack() as ctx:
            tc._emit_ctx = ctx
            _emit(tc, nc, masks, cls, bbox, out)
    nc.compile()
    return nc


_NC = None


def _get_program():
    global _NC
    if _NC is None:
        _NC = _build_program()
    return _NC


def make_in_maps(mask_output, class_indices, bbox_tensor):
    mask_output = np.asarray(mask_output, dtype=np.float32)
    class_indices = np.asarray(class_indices).astype(np.int32)
    bbox_tensor = np.asarray(bbox_tensor, dtype=np.float32)
    in_maps = []
    for cidx in range(N_CORES):
        sl = slice(cidx * N_LOC, (cidx + 1) * N_LOC)
        in_maps.append(
            {
                "masks": np.ascontiguousarray(mask_output[sl]),
                "cls": np.ascontiguousarray(class_indices[sl].reshape(N_LOC, 1)),
                "bbox": np.ascontiguousarray(bbox_tensor[sl]),
            }
        )
    return in_maps


def kernel(mask_output, class_indices, bbox_tensor, scene_h=H, scene_w=W, **kwargs):
    assert int(scene_h) == H and int(scene_w) == W
    from concourse.bass_utils import run_bass_kernel_spmd

    nc = _get_program()
    in_maps = make_in_maps(mask_output, class_indices, bbox_tensor)
    res = run_bass_kernel_spmd(nc, in_maps, list(range(N_CORES)))
    out = np.concatenate([r["out"] for r in res.results], axis=0)
    return out.astype(np.float32, copy=False)

